# revision 23
# baseline (speedup 1.0000x reference)
"""Trainium2 Bass kernel for KANEX attention (MLP -> qkv -> windowed causal attention -> out proj).

The second MLP linear is folded into the qkv projection on the host:
qkv = h@Wg.T + bg = a@(Wg@W2).T + (Wg@b2 + bg) where a = silu(x@W1.T + b1),
so the device only computes one 1024x1024 layer before qkv.

Sharding: tokens are split across 8 cores for the silu layer; a is AllGathered
in two feature-halves (bf16, the first gather overlapping the second half of the
layer); each core then computes q,k,v for its 2 heads over all tokens, runs
attention for (2 heads x 2 batches), and produces a partial output through its
128-column slice of Wo. Host sums the 8 bf16 partial outputs and adds bo.

Layout is transposed ([feature, token]) so every matmul chains without transposes.
v alone is produced directly in [token, feature] layout by using the a tile as the
matmul's stationary operand (out = a_tile.T @ W2gv.T), with a ones column appended
so the pv matmul also accumulates softmax sums. The two heads' QK matmuls use
disjoint 64-row groups of the PE array (concurrent via tile_position), write the
two halves of one 2-bank PSUM tile, and share a single [128,1024] Exp activation.
Mask j >= i + 64 is handled by skipping fully-masked 128-key chunks and
multiplying exp() by 0/1 masks on the 5 partially-masked diagonals.
"""

import numpy as np
import ml_dtypes

BF = ml_dtypes.bfloat16

N_CORES = 8
DIM = 1024
HEADS = 16
HEAD_DIM = 64
WINDOW = 64
B = 2
N = 2048
T = B * N            # 4096 tokens
TPC = T // N_CORES   # 512 tokens per core
HPC = HEADS // N_CORES  # 2 heads per core
KT = DIM // 128      # 8 k-tiles of the 1024 contraction
QB = 512             # query block (free dim)
KC = 128             # key chunk (partition dim)
NQB = N // QB        # 4 query blocks per batch
NKC = N // KC        # 16 key chunks per batch

LAST_RESULT = None   # BassKernelResults of the most recent run (for test harness)

_PROGRAM = None      # cached compiled Bass program


def _build_program():
    from concourse import bacc, mybir, tile

    f32 = mybir.dt.float32
    bf16 = mybir.dt.bfloat16
    AF = mybir.ActivationFunctionType

    nc = bacc.Bacc("TRN2", target_bir_lowering=False, debug=False,
                   num_devices=N_CORES)

    def inp(name, shape, dt):
        return nc.dram_tensor(name, shape, dt, kind="ExternalInput").ap()

    xT = inp("xT", [128, KT * TPC], bf16)             # [in-slice, k*512+t]
    w1 = inp("w1", [128, KT * KT * 128], bf16)        # (k,o) tiles of W1.T
    wgqk = inp("wgqk", [128, KT * 2 * 128], bf16)     # (k, og) tiles of (Wg@W2).T
    wgvT = inp("wgvT", [128, KT * 128], bf16)         # k tiles of (Wg_v@W2).T
    wo = inp("wo", [128, 1024], bf16)                 # [head-dim rows, 1024 out]
    b1d = inp("b1d", [128, KT], f32)
    bgqk = inp("bgqk", [128, 2], f32)
    bgvd = inp("bgvd", [128, 128], f32)               # bg_v broadcast along rows
    mkd = inp("mkd", [128, 5 * 2 * QB], bf16)         # 5 diagonal 0/1 masks (x2 heads)
    out_d = nc.dram_tensor("out", [DIM, T], bf16, kind="ExternalOutput").ap()

    HALF = KT // 2 * TPC                               # feature-half of h (2048 cols)

    with tile.TileContext(nc) as tc:
        with (
            tc.tile_pool(name="const", bufs=1) as cpool,
            tc.tile_pool(name="dram", bufs=1, space="DRAM") as dpool,
            tc.tile_pool(name="hstr", bufs=8) as hpool,
            tc.tile_pool(name="att", bufs=4) as apool,
            tc.tile_pool(name="attout", bufs=6) as opool,
            tc.tile_pool(name="fin", bufs=4) as fpool,
            tc.tile_pool(name="ps", bufs=2, space="PSUM") as psP,
        ):
            # ---- persistent constants in SBUF ----
            # DMA order = SP FIFO order: what the MLP needs first loads first
            x_sb = cpool.tile([128, KT * TPC], bf16)
            nc.sync.dma_start(out=x_sb[:, :], in_=xT[:, :])
            b1_sb = cpool.tile([128, KT], f32)
            nc.sync.dma_start(out=b1_sb[:, :], in_=b1d[:, :])
            w1_sb = cpool.tile([128, KT * KT * 128], bf16)
            nc.sync.dma_start(out=w1_sb[:, :], in_=w1[:, :])
            wgqk_sb = cpool.tile([128, KT * 2 * 128], bf16)
            nc.sync.dma_start(out=wgqk_sb[:, :], in_=wgqk[:, :])
            wgvT_sb = cpool.tile([128, KT * 128], bf16)
            nc.sync.dma_start(out=wgvT_sb[:, :], in_=wgvT[:, :])
            bgqk_sb = cpool.tile([128, 2], f32)
            nc.sync.dma_start(out=bgqk_sb[:, :], in_=bgqk[:, :])
            bgv_sb = cpool.tile([128, 128], f32)
            nc.sync.dma_start(out=bgv_sb[:, :], in_=bgvd[:, :])
            mk_sb = cpool.tile([128, 5 * 2 * QB], bf16)
            nc.sync.dma_start(out=mk_sb[:, :], in_=mkd[:, :])
            wo_sb = cpool.tile([128, 1024], bf16)
            nc.sync.dma_start(out=wo_sb[:, :], in_=wo[:, :])

            def w_tile(wsb, k, o, no):
                return wsb[:, (k * no + o) * 128:(k * no + o + 1) * 128]

            # ---- silu layer on own 512 tokens (transposed layout) ----
            a_sb = cpool.tile([128, KT * TPC], bf16)   # silu(x@W1.T+b1).T tiles
            a_in = [dpool.tile([128, HALF], bf16, name=f"a_in{i}") for i in range(2)]
            a_out = [dpool.tile([N_CORES, 128, HALF], bf16, addr_space="Shared",
                                name=f"a_out{i}") for i in range(2)]
            for half in range(2):                      # AG half 0 overlaps half 1
                for op in range(2 * half, 2 * half + 2):
                    ps = psP.tile([128, 2 * TPC], f32, tag="mm2", name=f"psL1_{op}")
                    for s in range(2):
                        o = 2 * op + s
                        for k in range(KT):
                            nc.tensor.matmul(ps[:, s * TPC:(s + 1) * TPC],
                                             w_tile(w1_sb, k, o, KT),
                                             x_sb[:, k * TPC:(k + 1) * TPC],
                                             start=(k == 0), stop=(k == KT - 1))
                    for s in range(2):
                        o = 2 * op + s
                        nc.scalar.activation(a_sb[:, o * TPC:(o + 1) * TPC],
                                             ps[:, s * TPC:(s + 1) * TPC],
                                             AF.Silu, bias=b1_sb[:, o:o + 1])
                nc.sync.dma_start(out=a_in[half][:, :],
                                  in_=a_sb[:, half * HALF:(half + 1) * HALF])
                nc.gpsimd.collective_compute(
                    "AllGather", mybir.AluOpType.bypass,
                    replica_groups=[list(range(N_CORES))],
                    ins=[a_in[half][:, :].opt()],
                    outs=[a_out[half][:, :, :].opt()])

            # ---- qkv for this core's 2 heads, all 4096 tokens ----
            # qT/kT: [128 = 2 heads x 64, 4096]; v: [token, feat] with ones cols
            qT_sb = cpool.tile([128, T], bf16)
            kT_sb = cpool.tile([128, T], bf16)
            v_sb = cpool.tile([128, (T // 128) * 130], bf16)
            # whole buffer to 1.0; data writes below leave the ones columns
            # (col 64 of each 65-wide block) intact
            nc.vector.memset(v_sb[:, :], 1.0)

            def qkv_pair(tp):                          # one token-block pair
                hts = []                               # [(tb0 lo, tb0 hi), (tb1 ...)]
                for s in range(2):
                    tb = 2 * tp + s
                    pair = []
                    for half in range(2):
                        ht = hpool.tile([128, HALF], bf16, tag="ht",
                                        name=f"ht_{tb}_{half}")
                        nc.sync.dma_start(out=ht[:, :], in_=a_out[half][tb, :, :])
                        pair.append(ht)
                    hts.append(pair)

                def h_slice(s, k, j0=0, jn=TPC):
                    ht = hts[s][0] if k < KT // 2 else hts[s][1]
                    kk = k % (KT // 2)
                    return ht[:, kk * TPC + j0:kk * TPC + j0 + jn]

                for og in range(2):                    # q, k for both blocks
                    ps = psP.tile([128, 2 * TPC], f32, tag="mm2",
                                  name=f"psqk_{tp}_{og}")
                    for s in range(2):
                        for k in range(KT):
                            nc.tensor.matmul(ps[:, s * TPC:(s + 1) * TPC],
                                             w_tile(wgqk_sb, k, og, 2),
                                             h_slice(s, k),
                                             start=(k == 0), stop=(k == KT - 1))
                    dst = qT_sb if og == 0 else kT_sb
                    # bias add on DVE: keeps ScalarE free for attention exps
                    nc.vector.tensor_scalar_add(
                        dst[:, 2 * tp * TPC:(2 * tp + 2) * TPC],
                        ps[:, :], bgqk_sb[:, og:og + 1])
                # v in [token, feat]: h tile is the stationary operand
                # (psv on the "mm2" tag: the long-lived attention pso
                # accumulators own "ps1"; v must not starve them)
                for s in range(2):
                    tb = 2 * tp + s
                    for j in range(TPC // 128):
                        psv = psP.tile([128, 128], f32, tag="mm2",
                                       name=f"psv_{tb}_{j}")
                        for k in range(KT):
                            nc.tensor.matmul(psv[:, :],
                                             h_slice(s, k, j * 128, 128),
                                             wgvT_sb[:, k * 128:(k + 1) * 128],
                                             start=(k == 0), stop=(k == KT - 1))
                        tj = tb * (TPC // 128) + j
                        for hh in range(HPC):
                            nc.vector.tensor_add(
                                v_sb[:, tj * 130 + hh * 65:tj * 130 + hh * 65 + 64],
                                psv[:, hh * 64:(hh + 1) * 64],
                                bgv_sb[:, hh * 64:(hh + 1) * 64])

            # ---- attention: 2 heads x 2 batches, windowed-causal ----
            def attn_block(beta, qb):
                if True:
                    nch = min(4 * qb + 5, NKC)
                    pso = [psP.tile([128, QB], f32, tag="ps1",
                                    name=f"psO_{beta}_{qb}_{i}")
                           for i in range(HPC)]
                    q0 = beta * N + qb * QB
                    for ci in range(nch):
                        k0 = beta * N + ci * KC
                        tj = k0 // 128
                        mi = ci - 4 * qb
                        # on diagonal chunk mi only queries f >= 128*mi-64 see
                        # any unmasked key: restrict QK/exp/PV to that range.
                        # head A packs at the end of psd bank 0, head B at the
                        # start of bank 1, so one contiguous Exp covers both.
                        off = max(0, 128 * mi - 64) if 0 <= mi <= 4 else 0
                        w = QB - off
                        psd = psP.tile([128, 2 * QB], f32, tag="mm2",
                                       name=f"psd_{beta}_{qb}_{ci}")
                        for hh in range(HPC):   # 64-row groups -> concurrent
                            col0 = QB - w if hh == 0 else QB
                            nc.tensor.matmul(
                                psd[:, col0:col0 + w],
                                kT_sb[hh * 64:(hh + 1) * 64, k0:k0 + KC],
                                qT_sb[hh * 64:(hh + 1) * 64,
                                      q0 + off:q0 + off + w],
                                start=True, stop=True)
                        pt = apool.tile([128, 2 * QB], bf16, tag="pt")
                        nc.scalar.activation(pt[:, 0:2 * w],
                                             psd[:, QB - w:QB + w], AF.Exp)
                        if 0 <= mi <= 4:
                            ptm = apool.tile([128, 2 * QB], bf16, tag="ptm")
                            nc.vector.tensor_mul(
                                ptm[:, 0:2 * w], pt[:, 0:2 * w],
                                mk_sb[:, mi * 2 * QB:mi * 2 * QB + 2 * w])
                            pt = ptm
                        for hh in range(HPC):
                            nc.tensor.matmul(
                                pso[hh][0:65, off:off + w],
                                v_sb[:, tj * 130 + hh * 65:tj * 130 + (hh + 1) * 65],
                                pt[:, hh * w:(hh + 1) * w],
                                start=(ci == 0), stop=(ci == nch - 1),
                                skip_group_check=True)
                    at = opool.tile([128, QB], bf16, tag="att",
                                    name=f"att_{beta}_{qb}")
                    for hh in range(HPC):
                        r = apool.tile([1, QB], f32, tag="recip")
                        nc.vector.reciprocal(r[:, :], pso[hh][64:65, :])
                        rb = apool.tile([64, QB], f32, tag="rb")
                        nc.gpsimd.partition_broadcast(rb[:, :], r[:, :])
                        nc.vector.tensor_mul(at[hh * 64:(hh + 1) * 64, :],
                                             pso[hh][0:64, :], rb[:, :])

                    # ---- partial out-projection for this query block ----
                    # last batch: ACT is idle by then, DVE is not; route the
                    # PSUM->SBUF copies accordingly
                    tcol = beta * N + qb * QB
                    for o in range(KT):
                        ps = psP.tile([128, QB], f32, tag="ps1",
                                      name=f"psF_{beta}_{qb}_{o}")
                        nc.tensor.matmul(ps[:, :], wo_sb[:, o * 128:(o + 1) * 128],
                                         at[:, :], start=True, stop=True)
                        ot = fpool.tile([128, QB], bf16, tag="fin")
                        if beta == B - 1:
                            nc.scalar.activation(ot[:, :], ps[:, :], AF.Identity)
                        else:
                            nc.vector.tensor_copy(ot[:, :], ps[:, :])
                        nc.sync.dma_start(
                            out=out_d[o * 128:(o + 1) * 128, tcol:tcol + QB],
                            in_=ot[:, :])

            # interleaved emission: each attention block is emitted as soon as
            # its token blocks' q/k/v exist, so exp work reaches ScalarE
            # continuously instead of queueing behind the full qkv loop.
            # attn(b,0) needs that batch's first two token blocks only.
            qkv_pair(0)                                # tb 0,1
            attn_block(0, 0)
            qkv_pair(2)                                # tb 4,5
            attn_block(1, 0)
            qkv_pair(1)                                # tb 2,3
            attn_block(0, 1)
            attn_block(0, 2)
            attn_block(0, 3)
            qkv_pair(3)                                # tb 6,7
            attn_block(1, 1)
            attn_block(1, 2)
            attn_block(1, 3)

    nc.compile()
    return nc


def _host_prep(x, W1, b1, W2, b2, Wg, bg, Wo, bo):
    X = np.ascontiguousarray(x.reshape(T, DIM))

    def tile_wT(W_T, no):
        # W_T: [1024 in, no*128 out] -> [128, (k, o) tiles]
        kt = W_T.shape[0] // 128
        t = W_T.reshape(kt, 128, no, 128).transpose(1, 0, 2, 3)
        return np.ascontiguousarray(t.reshape(128, kt * no * 128)).astype(BF)

    w1h = tile_wT(W1.T.astype(np.float32), KT)
    b1h = np.ascontiguousarray(b1.reshape(KT, 128).T).astype(np.float32)
    W2f = np.asarray(W2, dtype=np.float32)
    b2f = np.asarray(b2, dtype=np.float32)

    scale = HEAD_DIM ** -0.5
    # per-diagonal masks restricted to the live query range [off, 512), packed
    # [headA(w) | headB(w)] at column mi*1024 (matching the kernel's exp range)
    p = np.arange(128)[:, None]
    mkh_f = np.zeros((128, 5 * 2 * QB), dtype=np.float32)
    for mi in range(5):
        off = max(0, 128 * mi - 64)
        w = QB - off
        f = np.arange(off, QB)[None, :]
        m = (128 * mi + p - f <= WINDOW - 1).astype(np.float32)  # [128, w]
        mkh_f[:, mi * 2 * QB:mi * 2 * QB + w] = m
        mkh_f[:, mi * 2 * QB + w:mi * 2 * QB + 2 * w] = m
    mkh = np.ascontiguousarray(mkh_f).astype(BF)

    in_maps = []
    for c in range(N_CORES):
        xc = X[c * TPC:(c + 1) * TPC].T  # [1024, 512]
        xh = np.ascontiguousarray(
            xc.reshape(KT, 128, TPC).transpose(1, 0, 2).reshape(128, KT * TPC)
        ).astype(BF)
        heads = [HPC * c + i for i in range(HPC)]
        qrows = np.concatenate([np.arange(h * 64, (h + 1) * 64) for h in heads])
        # fold W2 into the qkv projection: qkv = a @ (Wg@W2).T + (Wg@b2 + bg)
        Wg_q = np.asarray(Wg[qrows, :], np.float32) * scale
        Wg_k = np.asarray(Wg[DIM + qrows, :], np.float32)
        Wg_v = np.asarray(Wg[2 * DIM + qrows, :], np.float32)
        W2g_q = Wg_q @ W2f
        W2g_k = Wg_k @ W2f
        W2g_v = Wg_v @ W2f
        WgT_c = np.concatenate([W2g_q, W2g_k], axis=0).T   # [1024, 256]
        wgqkh = tile_wT(WgT_c, 2)
        # W2g_v.T tiled: [128, k*128 + vfeat], tile k = W2g_v[:, k-slice].T
        wgvh = np.ascontiguousarray(
            W2g_v.T.reshape(KT, 128, 128)
            .transpose(1, 0, 2).reshape(128, KT * 128)).astype(BF)
        bgq2 = Wg_q @ b2f + np.asarray(bg[qrows], np.float32) * scale
        bgk2 = Wg_k @ b2f + np.asarray(bg[DIM + qrows], np.float32)
        bgv2 = Wg_v @ b2f + np.asarray(bg[2 * DIM + qrows], np.float32)
        bgqkh = np.stack([bgq2, bgk2], axis=1).astype(np.float32)  # [128, 2]
        bgvh = np.ascontiguousarray(
            np.tile(bgv2[None, :], (128, 1))).astype(np.float32)
        woh = np.ascontiguousarray(Wo[:, qrows].T).astype(BF)  # [128, 1024]
        in_maps.append({
            "xT": xh, "w1": w1h, "wgqk": wgqkh, "wgvT": wgvh,
            "wo": woh, "b1d": b1h, "bgqk": bgqkh, "bgvd": bgvh,
            "mkd": mkh,
        })
    return in_maps


def kernel(x, W1, b1, W2, b2, Wg, bg, Wo, bo):
    global _PROGRAM, LAST_RESULT
    import os
    from concourse.bass_utils import run_bass_kernel_spmd

    if _PROGRAM is None:
        _PROGRAM = _build_program()
    in_maps = _host_prep(x, W1, b1, W2, b2, Wg, bg, Wo, bo)
    trace = bool(int(os.environ.get("KERNEL_TRACE", "0")))
    res = run_bass_kernel_spmd(_PROGRAM, in_maps, list(range(N_CORES)),
                               trace=trace)
    LAST_RESULT = res
    outT = np.zeros((DIM, T), dtype=np.float32)
    for c in range(N_CORES):
        outT += res.results[c]["out"].astype(np.float32)
    out = outT.T + bo[None, :]
    return out.reshape(B, N, DIM).astype(np.float32)


# revision 24
# speedup vs baseline: 1.0149x; 1.0149x over previous
"""Trainium2 Bass kernel for KANEX attention (MLP -> qkv -> windowed causal attention -> out proj).

The second MLP linear is folded into the qkv projection on the host:
qkv = h@Wg.T + bg = a@(Wg@W2).T + (Wg@b2 + bg) where a = silu(x@W1.T + b1),
so the device only computes one 1024x1024 layer before qkv.

Sharding: tokens are split across 8 cores for the silu layer; a is AllGathered
in two feature-halves (bf16, the first gather overlapping the second half of the
layer); each core then computes q,k,v for its 2 heads over all tokens, runs
attention for (2 heads x 2 batches), and produces a partial output through its
128-column slice of Wo. Host sums the 8 bf16 partial outputs and adds bo.

Layout is transposed ([feature, token]) so every matmul chains without transposes.
v alone is produced directly in [token, feature] layout by using the a tile as the
matmul's stationary operand (out = a_tile.T @ W2gv.T), with a ones column appended
so the pv matmul also accumulates softmax sums. The two heads' QK matmuls use
disjoint 64-row groups of the PE array (concurrent via tile_position), write the
two halves of one 2-bank PSUM tile, and share a single [128,1024] Exp activation.
Mask j >= i + 64 is handled by skipping fully-masked 128-key chunks and
multiplying exp() by 0/1 masks on the 5 partially-masked diagonals.
"""

import numpy as np
import ml_dtypes

BF = ml_dtypes.bfloat16

N_CORES = 8
DIM = 1024
HEADS = 16
HEAD_DIM = 64
WINDOW = 64
B = 2
N = 2048
T = B * N            # 4096 tokens
TPC = T // N_CORES   # 512 tokens per core
HPC = HEADS // N_CORES  # 2 heads per core
KT = DIM // 128      # 8 k-tiles of the 1024 contraction
QB = 512             # query block (free dim)
KC = 128             # key chunk (partition dim)
NQB = N // QB        # 4 query blocks per batch
NKC = N // KC        # 16 key chunks per batch

LAST_RESULT = None   # BassKernelResults of the most recent run (for test harness)

_PROGRAM = None      # cached compiled Bass program


def _build_program():
    from concourse import bacc, mybir, tile

    f32 = mybir.dt.float32
    bf16 = mybir.dt.bfloat16
    AF = mybir.ActivationFunctionType

    nc = bacc.Bacc("TRN2", target_bir_lowering=False, debug=False,
                   num_devices=N_CORES)

    def inp(name, shape, dt):
        return nc.dram_tensor(name, shape, dt, kind="ExternalInput").ap()

    xT = inp("xT", [128, KT * TPC], bf16)             # [in-slice, k*512+t]
    w1 = inp("w1", [128, KT * KT * 128], bf16)        # (k,o) tiles of W1.T
    wgqk = inp("wgqk", [128, KT * 2 * 128], bf16)     # (k, og) tiles of (Wg@W2).T
    wgvT = inp("wgvT", [128, KT * 128], bf16)         # k tiles of (Wg_v@W2).T
    wo = inp("wo", [128, 1024], bf16)                 # [head-dim rows, 1024 out]
    b1d = inp("b1d", [128, KT], f32)
    bgqk = inp("bgqk", [128, 2], f32)
    bgvd = inp("bgvd", [128, 128], f32)               # bg_v broadcast along rows
    mkd = inp("mkd", [128, 5 * 2 * QB], bf16)         # 5 diagonal 0/1 masks (x2 heads)
    out_d = nc.dram_tensor("out", [DIM, T], bf16, kind="ExternalOutput").ap()

    HALF = KT // 2 * TPC                               # feature-half of h (2048 cols)

    with tile.TileContext(nc) as tc:
        with (
            tc.tile_pool(name="const", bufs=1) as cpool,
            tc.tile_pool(name="dram", bufs=1, space="DRAM") as dpool,
            tc.tile_pool(name="hstr", bufs=8) as hpool,
            tc.tile_pool(name="att", bufs=4) as apool,
            tc.tile_pool(name="attout", bufs=6) as opool,
            tc.tile_pool(name="fin", bufs=4) as fpool,
            tc.tile_pool(name="ps", bufs=2, space="PSUM") as psP,
        ):
            # ---- persistent constants in SBUF ----
            # DMA order = SP FIFO order: what the MLP needs first loads first
            x_sb = cpool.tile([128, KT * TPC], bf16)
            nc.sync.dma_start(out=x_sb[:, :], in_=xT[:, :])
            b1_sb = cpool.tile([128, KT], f32)
            nc.sync.dma_start(out=b1_sb[:, :], in_=b1d[:, :])
            w1_sb = cpool.tile([128, KT * KT * 128], bf16)
            nc.sync.dma_start(out=w1_sb[:, :], in_=w1[:, :])
            wgqk_sb = cpool.tile([128, KT * 2 * 128], bf16)
            nc.sync.dma_start(out=wgqk_sb[:, :], in_=wgqk[:, :])
            wgvT_sb = cpool.tile([128, KT * 128], bf16)
            nc.sync.dma_start(out=wgvT_sb[:, :], in_=wgvT[:, :])
            bgqk_sb = cpool.tile([128, 2], f32)
            nc.sync.dma_start(out=bgqk_sb[:, :], in_=bgqk[:, :])
            bgv_sb = cpool.tile([128, 128], f32)
            nc.sync.dma_start(out=bgv_sb[:, :], in_=bgvd[:, :])
            mk_sb = cpool.tile([128, 5 * 2 * QB], bf16)
            nc.sync.dma_start(out=mk_sb[:, :], in_=mkd[:, :])
            wo_sb = cpool.tile([128, 1024], bf16)
            nc.sync.dma_start(out=wo_sb[:, :], in_=wo[:, :])

            def w_tile(wsb, k, o, no):
                return wsb[:, (k * no + o) * 128:(k * no + o + 1) * 128]

            # ---- silu layer on own 512 tokens (transposed layout) ----
            a_sb = cpool.tile([128, KT * TPC], bf16)   # silu(x@W1.T+b1).T tiles
            a_in = [dpool.tile([128, HALF], bf16, name=f"a_in{i}") for i in range(2)]
            a_out = [dpool.tile([N_CORES, 128, HALF], bf16, addr_space="Shared",
                                name=f"a_out{i}") for i in range(2)]
            for half in range(2):                      # AG half 0 overlaps half 1
                for op in range(2 * half, 2 * half + 2):
                    ps = psP.tile([128, 2 * TPC], f32, tag="mm2", name=f"psL1_{op}")
                    for s in range(2):
                        o = 2 * op + s
                        for k in range(KT):
                            nc.tensor.matmul(ps[:, s * TPC:(s + 1) * TPC],
                                             w_tile(w1_sb, k, o, KT),
                                             x_sb[:, k * TPC:(k + 1) * TPC],
                                             start=(k == 0), stop=(k == KT - 1))
                    for s in range(2):
                        o = 2 * op + s
                        nc.scalar.activation(a_sb[:, o * TPC:(o + 1) * TPC],
                                             ps[:, s * TPC:(s + 1) * TPC],
                                             AF.Silu, bias=b1_sb[:, o:o + 1])
                nc.sync.dma_start(out=a_in[half][:, :],
                                  in_=a_sb[:, half * HALF:(half + 1) * HALF])
                nc.gpsimd.collective_compute(
                    "AllGather", mybir.AluOpType.bypass,
                    replica_groups=[list(range(N_CORES))],
                    ins=[a_in[half][:, :].opt()],
                    outs=[a_out[half][:, :, :].opt()])

            # ---- qkv for this core's 2 heads, all 4096 tokens ----
            # qT/kT: [128 = 2 heads x 64, 4096]; v: [token, feat] with ones cols
            qT_sb = cpool.tile([128, T], bf16)
            kT_sb = cpool.tile([128, T], bf16)
            v_sb = cpool.tile([128, (T // 128) * 130], bf16)
            # whole buffer to 1.0; data writes below leave the ones columns
            # (col 64 of each 65-wide block) intact
            nc.vector.memset(v_sb[:, :], 1.0)

            def qkv_pair(tp):                          # one token-block pair
                hts = []                               # [(tb0 lo, tb0 hi), (tb1 ...)]
                for s in range(2):
                    tb = 2 * tp + s
                    pair = []
                    for half in range(2):
                        ht = hpool.tile([128, HALF], bf16, tag="ht",
                                        name=f"ht_{tb}_{half}")
                        nc.sync.dma_start(out=ht[:, :], in_=a_out[half][tb, :, :])
                        pair.append(ht)
                    hts.append(pair)

                def h_slice(s, k, j0=0, jn=TPC):
                    ht = hts[s][0] if k < KT // 2 else hts[s][1]
                    kk = k % (KT // 2)
                    return ht[:, kk * TPC + j0:kk * TPC + j0 + jn]

                for og in range(2):                    # q, k for both blocks
                    ps = psP.tile([128, 2 * TPC], f32, tag="mm2",
                                  name=f"psqk_{tp}_{og}")
                    for s in range(2):
                        for k in range(KT):
                            nc.tensor.matmul(ps[:, s * TPC:(s + 1) * TPC],
                                             w_tile(wgqk_sb, k, og, 2),
                                             h_slice(s, k),
                                             start=(k == 0), stop=(k == KT - 1))
                    dst = qT_sb if og == 0 else kT_sb
                    # bias add on DVE: keeps ScalarE free for attention exps
                    nc.vector.tensor_scalar_add(
                        dst[:, 2 * tp * TPC:(2 * tp + 2) * TPC],
                        ps[:, :], bgqk_sb[:, og:og + 1])
                # v in [token, feat]: h tile is the stationary operand
                # (psv on the "mm2" tag: the long-lived attention pso
                # accumulators own "ps1"; v must not starve them)
                for s in range(2):
                    tb = 2 * tp + s
                    for j in range(TPC // 128):
                        psv = psP.tile([128, 128], f32, tag="mm2",
                                       name=f"psv_{tb}_{j}")
                        for k in range(KT):
                            nc.tensor.matmul(psv[:, :],
                                             h_slice(s, k, j * 128, 128),
                                             wgvT_sb[:, k * 128:(k + 1) * 128],
                                             start=(k == 0), stop=(k == KT - 1))
                        tj = tb * (TPC // 128) + j
                        for hh in range(HPC):
                            nc.vector.tensor_add(
                                v_sb[:, tj * 130 + hh * 65:tj * 130 + hh * 65 + 64],
                                psv[:, hh * 64:(hh + 1) * 64],
                                bgv_sb[:, hh * 64:(hh + 1) * 64])

            # ---- attention: 2 heads x 2 batches, windowed-causal ----
            def attn_block(beta, qb):
                if True:
                    nch = min(4 * qb + 5, NKC)
                    pso = [psP.tile([128, QB], f32, tag="ps1",
                                    name=f"psO_{beta}_{qb}_{i}")
                           for i in range(HPC)]
                    q0 = beta * N + qb * QB
                    for ci in range(nch):
                        k0 = beta * N + ci * KC
                        tj = k0 // 128
                        mi = ci - 4 * qb
                        # on diagonal chunk mi only queries f >= 128*mi-64 see
                        # any unmasked key: restrict QK/exp/PV to that range.
                        # head A packs at the end of psd bank 0, head B at the
                        # start of bank 1, so one contiguous Exp covers both.
                        off = max(0, 128 * mi - 64) if 0 <= mi <= 4 else 0
                        w = QB - off
                        psd = psP.tile([128, 2 * QB], f32, tag="mm2",
                                       name=f"psd_{beta}_{qb}_{ci}")
                        for hh in range(HPC):   # 64-row groups -> concurrent
                            col0 = QB - w if hh == 0 else QB
                            nc.tensor.matmul(
                                psd[:, col0:col0 + w],
                                kT_sb[hh * 64:(hh + 1) * 64, k0:k0 + KC],
                                qT_sb[hh * 64:(hh + 1) * 64,
                                      q0 + off:q0 + off + w],
                                start=True, stop=True)
                        pt = apool.tile([128, 2 * QB], bf16, tag="pt")
                        nc.scalar.activation(pt[:, 0:2 * w],
                                             psd[:, QB - w:QB + w], AF.Exp)
                        if 0 <= mi <= 4:
                            ptm = apool.tile([128, 2 * QB], bf16, tag="ptm")
                            nc.vector.tensor_mul(
                                ptm[:, 0:2 * w], pt[:, 0:2 * w],
                                mk_sb[:, mi * 2 * QB:mi * 2 * QB + 2 * w])
                            pt = ptm
                        for hh in range(HPC):
                            nc.tensor.matmul(
                                pso[hh][0:65, off:off + w],
                                v_sb[:, tj * 130 + hh * 65:tj * 130 + (hh + 1) * 65],
                                pt[:, hh * w:(hh + 1) * w],
                                start=(ci == 0), stop=(ci == nch - 1),
                                skip_group_check=True)
                    at = opool.tile([128, QB], bf16, tag="att",
                                    name=f"att_{beta}_{qb}")
                    for hh in range(HPC):
                        r = apool.tile([1, QB], f32, tag="recip")
                        nc.vector.reciprocal(r[:, :], pso[hh][64:65, :])
                        rb = apool.tile([64, QB], f32, tag="rb")
                        nc.gpsimd.partition_broadcast(rb[:, :], r[:, :])
                        nc.vector.tensor_mul(at[hh * 64:(hh + 1) * 64, :],
                                             pso[hh][0:64, :], rb[:, :])

                    # ---- partial out-projection for this query block ----
                    # last batch: ACT is idle by then, DVE is not; route the
                    # PSUM->SBUF copies accordingly
                    tcol = beta * N + qb * QB
                    for o in range(KT):
                        ps = psP.tile([128, QB], f32, tag="ps1",
                                      name=f"psF_{beta}_{qb}_{o}")
                        nc.tensor.matmul(ps[:, :], wo_sb[:, o * 128:(o + 1) * 128],
                                         at[:, :], start=True, stop=True)
                        ot = fpool.tile([128, QB], bf16, tag="fin")
                        if beta == B - 1:
                            nc.scalar.activation(ot[:, :], ps[:, :], AF.Identity)
                        else:
                            nc.vector.tensor_copy(ot[:, :], ps[:, :])
                        nc.sync.dma_start(
                            out=out_d[o * 128:(o + 1) * 128, tcol:tcol + QB],
                            in_=ot[:, :])

            # interleaved emission: each attention block is emitted as soon as
            # its token blocks' q/k/v exist, so exp work reaches ScalarE
            # continuously instead of queueing behind the full qkv loop.
            # attn(b,0) needs that batch's first two token blocks only.
            qkv_pair(0)                                # tb 0,1
            attn_block(0, 0)
            qkv_pair(2)                                # tb 4,5
            attn_block(1, 0)
            # within each group: biggest block (most key chunks) first, so its
            # long QK->exp->PV chain overlaps the others; smallest is the tail
            qkv_pair(1)                                # tb 2,3
            attn_block(0, 3)
            attn_block(0, 2)
            attn_block(0, 1)
            qkv_pair(3)                                # tb 6,7
            attn_block(1, 3)
            attn_block(1, 2)
            attn_block(1, 1)

    nc.compile()
    return nc


def _host_prep(x, W1, b1, W2, b2, Wg, bg, Wo, bo):
    X = np.ascontiguousarray(x.reshape(T, DIM))

    def tile_wT(W_T, no):
        # W_T: [1024 in, no*128 out] -> [128, (k, o) tiles]
        kt = W_T.shape[0] // 128
        t = W_T.reshape(kt, 128, no, 128).transpose(1, 0, 2, 3)
        return np.ascontiguousarray(t.reshape(128, kt * no * 128)).astype(BF)

    w1h = tile_wT(W1.T.astype(np.float32), KT)
    b1h = np.ascontiguousarray(b1.reshape(KT, 128).T).astype(np.float32)
    W2f = np.asarray(W2, dtype=np.float32)
    b2f = np.asarray(b2, dtype=np.float32)

    scale = HEAD_DIM ** -0.5
    # per-diagonal masks restricted to the live query range [off, 512), packed
    # [headA(w) | headB(w)] at column mi*1024 (matching the kernel's exp range)
    p = np.arange(128)[:, None]
    mkh_f = np.zeros((128, 5 * 2 * QB), dtype=np.float32)
    for mi in range(5):
        off = max(0, 128 * mi - 64)
        w = QB - off
        f = np.arange(off, QB)[None, :]
        m = (128 * mi + p - f <= WINDOW - 1).astype(np.float32)  # [128, w]
        mkh_f[:, mi * 2 * QB:mi * 2 * QB + w] = m
        mkh_f[:, mi * 2 * QB + w:mi * 2 * QB + 2 * w] = m
    mkh = np.ascontiguousarray(mkh_f).astype(BF)

    in_maps = []
    for c in range(N_CORES):
        xc = X[c * TPC:(c + 1) * TPC].T  # [1024, 512]
        xh = np.ascontiguousarray(
            xc.reshape(KT, 128, TPC).transpose(1, 0, 2).reshape(128, KT * TPC)
        ).astype(BF)
        heads = [HPC * c + i for i in range(HPC)]
        qrows = np.concatenate([np.arange(h * 64, (h + 1) * 64) for h in heads])
        # fold W2 into the qkv projection: qkv = a @ (Wg@W2).T + (Wg@b2 + bg)
        Wg_q = np.asarray(Wg[qrows, :], np.float32) * scale
        Wg_k = np.asarray(Wg[DIM + qrows, :], np.float32)
        Wg_v = np.asarray(Wg[2 * DIM + qrows, :], np.float32)
        W2g_q = Wg_q @ W2f
        W2g_k = Wg_k @ W2f
        W2g_v = Wg_v @ W2f
        WgT_c = np.concatenate([W2g_q, W2g_k], axis=0).T   # [1024, 256]
        wgqkh = tile_wT(WgT_c, 2)
        # W2g_v.T tiled: [128, k*128 + vfeat], tile k = W2g_v[:, k-slice].T
        wgvh = np.ascontiguousarray(
            W2g_v.T.reshape(KT, 128, 128)
            .transpose(1, 0, 2).reshape(128, KT * 128)).astype(BF)
        bgq2 = Wg_q @ b2f + np.asarray(bg[qrows], np.float32) * scale
        bgk2 = Wg_k @ b2f + np.asarray(bg[DIM + qrows], np.float32)
        bgv2 = Wg_v @ b2f + np.asarray(bg[2 * DIM + qrows], np.float32)
        bgqkh = np.stack([bgq2, bgk2], axis=1).astype(np.float32)  # [128, 2]
        bgvh = np.ascontiguousarray(
            np.tile(bgv2[None, :], (128, 1))).astype(np.float32)
        woh = np.ascontiguousarray(Wo[:, qrows].T).astype(BF)  # [128, 1024]
        in_maps.append({
            "xT": xh, "w1": w1h, "wgqk": wgqkh, "wgvT": wgvh,
            "wo": woh, "b1d": b1h, "bgqk": bgqkh, "bgvd": bgvh,
            "mkd": mkh,
        })
    return in_maps


def kernel(x, W1, b1, W2, b2, Wg, bg, Wo, bo):
    global _PROGRAM, LAST_RESULT
    import os
    from concourse.bass_utils import run_bass_kernel_spmd

    if _PROGRAM is None:
        _PROGRAM = _build_program()
    in_maps = _host_prep(x, W1, b1, W2, b2, Wg, bg, Wo, bo)
    trace = bool(int(os.environ.get("KERNEL_TRACE", "0")))
    res = run_bass_kernel_spmd(_PROGRAM, in_maps, list(range(N_CORES)),
                               trace=trace)
    LAST_RESULT = res
    outT = np.zeros((DIM, T), dtype=np.float32)
    for c in range(N_CORES):
        outT += res.results[c]["out"].astype(np.float32)
    out = outT.T + bo[None, :]
    return out.reshape(B, N, DIM).astype(np.float32)
